# revision 3
# baseline (speedup 1.0000x reference)
"""EnergyAttention Trainium2 kernel (8 NeuronCores, head-sharded).

Strategy: shard the 16 heads across 8 cores (2 heads per core). Each core:
  - projects K^T (bf16, zero-padded per head), K-natural (fp8e4m3, padded to
    96 cols: 64 K + tens column + zeros) and Q^T (float32r)
  - runs 5 energy steps with transposed scores S^T[k, q]:
      * scores: bf16 matmuls (full-128 zero-padded contraction)
      * exp: split across ACT (native exp -> fp8 out) and DVE (one-instruction
        Schraudolph bit trick: round(x*8*log2e + 8*(7+delta)) as int8 IS the
        fp8e4m3 bit pattern of exp(x))
      * grad: fp8e4m3 DoubleRow matmuls (two 128-k chunks per instruction,
        2x PE rate); denominator rides col 64 of the padded kones (10.0
        column folds step_size=0.1 into the reciprocal)
      * update: DVE reciprocal+mul (PSUM-side), gpsimd broadcast/add/copies
  - output projection through Wo columns in float32r, bf16 partial out
Host: transposes/casts inputs, sums the 8 bf16 partial outputs in fp32.
"""

import numpy as np
import ml_dtypes

BF16 = ml_dtypes.bfloat16

N_CORES = 8
D = 1024
K = 4096
Q = 2048
H = 16
HD = 64
STEPS = 5
STEP_SIZE = 0.1
BETA = 1.0 / np.sqrt(np.float32(HD))  # 1/8

QB = 512
LOG2E = 1.4426950408889634
SCH_DELTA = -0.0580  # tuned: mean-zero relative error of the bit-trick exp

# exp tile engine schedule per q-block: 32 k-chunks -> ACT or DVE.
# ACT is a bit faster per tile; DVE also does recip/mul side work.
N_DVE_TILES = 14
_dve_set = frozenset(
    int(round(i * 32.0 / N_DVE_TILES)) % 32 for i in range(N_DVE_TILES)
)

_CACHE = {}


def build_program(d=D, k=K, q=Q, steps=STEPS, n_cores=N_CORES):
    """Build + compile the per-core Bass program. Returns the Bacc object."""
    from contextlib import ExitStack

    import concourse.tile as tile
    from concourse import bacc, mybir

    f32 = mybir.dt.float32
    f32r = mybir.dt.float32r
    bf16 = mybir.dt.bfloat16
    fp8 = mybir.dt.float8e4
    i8 = mybir.dt.int8

    ndc = d // 128       # D chunks (contraction for projections)
    nkb = k // 512       # k blocks for K^T projection
    nkc = k // 128       # k chunks for the step loop
    nqb = q // QB        # q blocks
    beta = float(1.0 / np.sqrt(np.float64(HD)))
    sch_c1 = beta * 8.0 * LOG2E
    sch_c2 = 8.0 * (7.0 + SCH_DELTA)

    nc = bacc.Bacc("TRN2", target_bir_lowering=False, debug=False,
                   num_devices=n_cores)
    ctxT = nc.dram_tensor("ctxT", [d, k], bf16, kind="ExternalInput").ap()
    tgtT = nc.dram_tensor("tgtT", [d, q], f32r, kind="ExternalInput").ap()
    wk = nc.dram_tensor("wk", [d, 128], bf16, kind="ExternalInput").ap()
    wq = nc.dram_tensor("wq", [d, 128], f32r, kind="ExternalInput").ap()
    woT = nc.dram_tensor("woT", [128, d], f32r, kind="ExternalInput").ap()
    out = nc.dram_tensor("out", [q, d], bf16, kind="ExternalOutput").ap()

    EXP = mybir.ActivationFunctionType.Exp
    MULT = mybir.AluOpType.mult
    ADD = mybir.AluOpType.add

    with tile.TileContext(nc) as tc, ExitStack() as ctx:
        # ---------------- persistent pools ----------------
        kt_pool = ctx.enter_context(tc.tile_pool(name="kt", bufs=1))
        kon_pool = ctx.enter_context(tc.tile_pool(name="kones", bufs=1))
        qt_pool = ctx.enter_context(tc.tile_pool(name="qt", bufs=2 * nqb))
        qtb_pool = ctx.enter_context(tc.tile_pool(name="qtb", bufs=2 * nqb))
        w_pool = ctx.enter_context(tc.tile_pool(name="w", bufs=1))

        # per-head padded K^T: other head's rows zeroed -> full-128 contraction
        ktp = [kt_pool.tile([128, k], bf16, tag=f"ktp{h}", name=f"ktp{h}")
               for h in range(2)]
        nc.vector.memset(ktp[0][64:128, :], 0.0)
        nc.vector.memset(ktp[1][0:64, :], 0.0)
        # K natural fp8, padded to 96: cols 0..63 = K chunk, col 64 = 10.0
        # (denominator), cols 65..95 = 0 (DoubleRow needs M % 32 == 0).
        kones = [kon_pool.tile([128, nkc, 96], fp8, tag=f"kones{h}",
                               name=f"kones{h}")
                 for h in range(2)]
        for h in range(2):
            nc.vector.memset(kones[h][:, :, 64:96], 0.0)
            nc.gpsimd.memset(kones[h][:, :, 64:65], 10.0)
        wk_sb = w_pool.tile([128, d], bf16, tag="wk")
        wq_sb = w_pool.tile([128, d], f32r, tag="wq")
        wo_sb = w_pool.tile([128, d], f32r, tag="wo")

        for c in range(ndc):
            cs = slice(c * 128, (c + 1) * 128)
            nc.sync.dma_start(out=wk_sb[:, cs], in_=wk[cs, :])
            nc.sync.dma_start(out=wq_sb[:, cs], in_=wq[cs, :])
        nc.sync.dma_start(out=wo_sb[:], in_=woT[:])

        qt_tiles = []
        qtb_tiles = []

        # ---------------- phase A: projections ----------------
        with tc.tile_pool(name="ctxp", bufs=ndc) as ctx_pool, \
             tc.tile_pool(name="tgtp", bufs=ndc) as tgt_pool, \
             tc.tile_pool(name="psA", bufs=2, space="PSUM") as psA, \
             tc.tile_pool(name="psB", bufs=2, space="PSUM") as psB, \
             tc.tile_pool(name="psQ", bufs=2, space="PSUM") as psQ:
            ctx_tiles = [ctx_pool.tile([128, k], bf16, tag="ctx", name=f"ctx{c}")
                         for c in range(ndc)]
            tgt_tiles = [tgt_pool.tile([128, q], f32r, tag="tgt", name=f"tgt{c}")
                         for c in range(ndc)]
            for c in range(ndc):
                cs = slice(c * 128, (c + 1) * 128)
                nc.sync.dma_start(out=ctx_tiles[c][:], in_=ctxT[cs, :])
                nc.sync.dma_start(out=tgt_tiles[c][:], in_=tgtT[cs, :])

            # K^T = Wk_pair^T @ context^T  (bf16)
            for kb in range(nkb):
                ks = slice(kb * 512, (kb + 1) * 512)
                pk = psA.tile([128, 512], f32, tag="pk")
                for c in range(ndc):
                    cs = slice(c * 128, (c + 1) * 128)
                    nc.tensor.matmul(out=pk[:], lhsT=wk_sb[:, cs],
                                     rhs=ctx_tiles[c][:, ks],
                                     start=(c == 0), stop=(c == ndc - 1))
                nc.vector.tensor_copy(out=ktp[0][0:64, ks], in_=pk[0:64, :])
                nc.vector.tensor_copy(out=ktp[1][64:128, ks], in_=pk[64:128, :])

            # K natural (both heads side by side), fp8 into kones
            for kc in range(nkc):
                ks = slice(kc * 128, (kc + 1) * 128)
                pn = psB.tile([128, 128], f32, tag="pn")
                for c in range(ndc):
                    cs = slice(c * 128, (c + 1) * 128)
                    nc.tensor.matmul(out=pn[:], lhsT=ctx_tiles[c][:, ks],
                                     rhs=wk_sb[:, cs],
                                     start=(c == 0), stop=(c == ndc - 1))
                for h in range(2):
                    nc.vector.tensor_copy(
                        out=kones[h][:, kc, 0:64],
                        in_=pn[:, h * 64:(h + 1) * 64])

            # Q^T projection in float32r (1 cyc/col at N=512)
            for j in range(nqb):
                qs = slice(j * QB, (j + 1) * QB)
                pq = psQ.tile([128, QB], f32, tag="pq")
                for c in range(ndc):
                    cs = slice(c * 128, (c + 1) * 128)
                    nc.tensor.matmul(out=pq[:], lhsT=wq_sb[:, cs],
                                     rhs=tgt_tiles[c][:, qs],
                                     start=(c == 0), stop=(c == ndc - 1))
                q0 = qt_pool.tile([128, QB], f32r, tag="qt")
                nc.vector.tensor_copy(out=q0[:], in_=pq[:])
                qb0 = qtb_pool.tile([128, QB], bf16, tag="qtb")
                nc.scalar.copy(out=qb0[:], in_=pq[:])
                qt_tiles.append(q0)
                qtb_tiles.append(qb0)

        # ---------------- phase B: energy steps ----------------
        with tc.tile_pool(name="pdrp", bufs=1) as pdr_pool, \
             tc.tile_pool(name="upd", bufs=8) as upd_pool, \
             tc.tile_pool(name="ps_s", bufs=3, space="PSUM") as ps_s, \
             tc.tile_pool(name="ps_g", bufs=1, space="PSUM") as ps_g:
            # p tiles: [128, kc, (h, q)] fp8, one q-block at a time
            pdr = pdr_pool.tile([128, nkc, 2, QB], fp8, tag="pdr")
            for t in range(steps):
                new_qt = []
                new_qtb = []
                for j in range(nqb):
                    qcur = qt_tiles[j]
                    qbcur = qtb_tiles[j]
                    gt = ps_g.tile([96, 2, QB], f32, tag="g",
                                   name=f"g{t}_{j}")
                    for kc in range(nkc):
                        s = ps_s.tile([128, 2 * QB], f32, tag="s")
                        for h in range(2):
                            nc.tensor.matmul(
                                out=s[:, h * QB:(h + 1) * QB],
                                lhsT=ktp[h][:, kc * 128:(kc + 1) * 128],
                                rhs=qbcur[:, :],
                                start=True, stop=True)
                        pslice = pdr[:, kc, :, :]
                        if kc in _dve_set:
                            nc.vector.tensor_scalar(
                                out=pslice.bitcast(i8), in0=s[:],
                                scalar1=sch_c1, scalar2=sch_c2,
                                op0=MULT, op1=ADD)
                        else:
                            nc.scalar.activation(pslice, s[:], EXP, scale=beta)
                        if kc % 2 == 1:
                            tpair = kc // 2
                            for h in range(2):
                                nc.tensor.matmul(
                                    out=gt[:, h, :],
                                    lhsT=kones[h][:, kc - 1:kc + 1, :],
                                    rhs=pdr[:, kc - 1:kc + 1, h, :],
                                    start=(tpair == 0), stop=(tpair == 15),
                                    perf_mode=mybir.MatmulPerfMode.DoubleRow)
                    # q update: q += (G/10) / (denom/10) == q + 0.1*G/denom
                    qn = qt_pool.tile([128, QB], f32r, tag="qt")
                    tm = upd_pool.tile([128, QB], f32, tag="tm")
                    for h in range(2):
                        hs = slice(h * 64, (h + 1) * 64)
                        # reciprocal lands on partition 0: partition_broadcast
                        # only reads correctly from a partition-0 source on HW
                        r = upd_pool.tile([1, QB], f32, tag="r")
                        nc.vector.reciprocal(out=r[:], in_=gt[64:65, h, :])
                        rb = upd_pool.tile([64, QB], f32, tag="rb")
                        nc.gpsimd.partition_broadcast(rb[:], r[0:1, :])
                        nc.vector.tensor_mul(out=tm[hs, :], in0=gt[0:64, h, :],
                                             in1=rb[:])
                    nc.gpsimd.tensor_add(out=qn[:], in0=qcur[:], in1=tm[:])
                    qb_new = qtb_pool.tile([128, QB], bf16, tag="qtb")
                    nc.gpsimd.tensor_copy(out=qb_new[:], in_=qn[:])
                    new_qt.append(qn)
                    new_qtb.append(qb_new)
                qt_tiles = new_qt
                qtb_tiles = new_qtb

        # ---------------- phase C: output projection (float32r) ----------------
        with tc.tile_pool(name="fo", bufs=6) as fo_pool, \
             tc.tile_pool(name="psO", bufs=4, space="PSUM") as psO:
            dob = min(512, d)
            for qb128 in range(q // 128):
                jt = qt_tiles[(qb128 * 128) // QB]
                qs = slice((qb128 * 128) % QB, (qb128 * 128) % QB + 128)
                for db in range(d // dob):
                    ds_ = slice(db * dob, (db + 1) * dob)
                    po = psO.tile([128, dob], f32, tag="po")
                    nc.tensor.matmul(out=po[:], lhsT=jt[:, qs],
                                     rhs=wo_sb[:, ds_],
                                     start=True, stop=True)
                    ot = fo_pool.tile([128, dob], bf16, tag="ot")
                    # alternate evacuation engines: ACT is idle in phase C
                    if db % 2 == 0:
                        nc.vector.tensor_copy(out=ot[:], in_=po[:])
                    else:
                        nc.scalar.copy(out=ot[:], in_=po[:])
                    nc.sync.dma_start(
                        out=out[qb128 * 128:(qb128 + 1) * 128, ds_],
                        in_=ot[:])

    nc.compile()
    return nc


def _get_program():
    if "nc" not in _CACHE:
        _CACHE["nc"] = build_program()
    return _CACHE["nc"]


def make_in_maps(context, target_init, Wq, Wk, Wo):
    """Host-side sharding/layout prep: one input map per core."""
    ctxT = np.ascontiguousarray(context.T).astype(BF16)        # [D, K]
    tgtT = np.ascontiguousarray(target_init.T.astype(np.float32))  # [D, Q]
    in_maps = []
    for c in range(N_CORES):
        h0, h1 = 2 * c, 2 * c + 1
        wk_c = np.concatenate([Wk[h0].T, Wk[h1].T], axis=1)    # [D, 128]
        wq_c = np.concatenate([Wq[h0].T, Wq[h1].T], axis=1)    # [D, 128]
        woT_c = np.ascontiguousarray(Wo[:, 128 * c:128 * (c + 1)].T)  # [128, D]
        in_maps.append({
            "ctxT": ctxT,
            "tgtT": tgtT,
            "wk": np.ascontiguousarray(wk_c).astype(BF16),
            "wq": np.ascontiguousarray(wq_c.astype(np.float32)),
            "woT": woT_c.astype(np.float32),
        })
    return in_maps


def kernel(context, target_init, Wq, Wk, Wo):
    context = np.asarray(context, dtype=np.float32)
    target_init = np.asarray(target_init, dtype=np.float32)
    Wq = np.asarray(Wq, dtype=np.float32)
    Wk = np.asarray(Wk, dtype=np.float32)
    Wo = np.asarray(Wo, dtype=np.float32)

    in_maps = make_in_maps(context, target_init, Wq, Wk, Wo)

    last_err = None
    for _attempt in range(3):
        try:
            results = _run_spmd(in_maps)
            break
        except Exception as e:  # transient axon RESOURCE_EXHAUSTED etc.
            last_err = e
            _CACHE.clear()
    else:
        raise last_err

    acc = np.zeros((Q, D), dtype=np.float32)
    for c in range(N_CORES):
        acc += results[c]["out"].astype(np.float32)
    return acc


def _run_spmd(in_maps):
    """Run the program on cores 0..7. Uses a cached jitted executable with
    device-resident zero buffers; falls back to run_bass_kernel_spmd."""
    nc = _get_program()
    try:
        runner = _CACHE.get("runner")
        if runner is None:
            runner = _SpmdRunner(nc, N_CORES)
            _CACHE["runner"] = runner
        return runner.run(in_maps)
    except Exception:
        _CACHE.pop("runner", None)
        from concourse.bass_utils import run_bass_kernel_spmd
        res = run_bass_kernel_spmd(nc, in_maps, list(range(N_CORES)))
        return res.results


class _SpmdRunner:
    """Persistent jitted shard_map executable (mirrors
    bass2jax.run_bass_via_pjrt's multi-core path, without output donation so
    the executable and zero buffers are reusable across calls)."""

    def __init__(self, nc, n_cores):
        import jax
        from jax.experimental.shard_map import shard_map
        from jax.sharding import Mesh, NamedSharding, PartitionSpec
        import concourse.mybir as mybir
        from concourse.bass2jax import (
            _bass_exec_p, install_neuronx_cc_hook, partition_id_tensor)

        install_neuronx_cc_hook()
        self.jax = jax
        self.n_cores = n_cores
        partition_name = (nc.partition_id_tensor.name
                          if nc.partition_id_tensor else None)
        in_names, out_names, out_avals, zero_outs = [], [], [], []
        for alloc in nc.m.functions[0].allocations:
            if not isinstance(alloc, mybir.MemoryLocationSet):
                continue
            name = alloc.memorylocations[0].name
            if alloc.kind == "ExternalInput":
                if name != partition_name:
                    in_names.append(name)
            elif alloc.kind == "ExternalOutput":
                shape = tuple(alloc.tensor_shape)
                dtype = mybir.dt.np(alloc.dtype)
                out_names.append(name)
                out_avals.append(jax.core.ShapedArray(shape, dtype))
                zero_outs.append(np.zeros(shape, dtype))
        self.in_names = in_names
        self.out_names = out_names
        self.out_avals = out_avals
        all_in_names = in_names + out_names
        if partition_name is not None:
            all_in_names.append(partition_name)

        def _body(*args):
            operands = list(args)
            if partition_name is not None:
                operands.append(partition_id_tensor())
            outs = _bass_exec_p.bind(
                *operands,
                out_avals=tuple(out_avals),
                in_names=tuple(all_in_names),
                out_names=tuple(out_names),
                lowering_input_output_aliases=(),
                sim_require_finite=True,
                sim_require_nnan=True,
                nc=nc,
            )
            return tuple(outs)

        devices = jax.devices()[:n_cores]
        mesh = Mesh(np.asarray(devices), ("core",))
        in_specs = (PartitionSpec("core"),) * (len(in_names) + len(out_names))
        out_specs = (PartitionSpec("core"),) * len(out_names)
        self.fn = jax.jit(
            shard_map(_body, mesh=mesh, in_specs=in_specs,
                      out_specs=out_specs, check_rep=False),
            keep_unused=True,
        )
        self.sharding = NamedSharding(mesh, PartitionSpec("core"))
        self.zeros_placed = [
            jax.device_put(np.concatenate([z] * n_cores, axis=0), self.sharding)
            for z in zero_outs
        ]

    def place(self, in_maps):
        concat = [
            np.concatenate([np.asarray(in_maps[c][n])
                            for c in range(self.n_cores)], axis=0)
            for n in self.in_names
        ]
        return [self.jax.device_put(a, self.sharding) for a in concat]

    def exec_placed(self, placed):
        outs = self.fn(*placed, *self.zeros_placed)
        self.jax.block_until_ready(outs)
        return outs

    def run(self, in_maps):
        outs = self.exec_placed(self.place(in_maps))
        per_core = []
        for c in range(self.n_cores):
            d = {}
            for i, n in enumerate(self.out_names):
                full = np.asarray(outs[i])
                sh = self.out_avals[i].shape
                d[n] = full.reshape(self.n_cores, *sh)[c]
            per_core.append(d)
        return per_core


# revision 13
# speedup vs baseline: 1.0516x; 1.0516x over previous
"""EnergyAttention Trainium2 kernel (8 NeuronCores, head-sharded).

Strategy: shard the 16 heads across 8 cores (2 heads per core). Each core:
  - projects K^T (bf16, zero-padded per head), K-natural (fp8e4m3, padded to
    96 cols: 64 K + tens column + zeros) and Q^T (float32r)
  - runs 5 energy steps with transposed scores S^T[k, q]:
      * scores: bf16 matmuls (full-128 zero-padded contraction)
      * exp on ACT (bf16 out, per-kc-pair p tiles so consumers pipeline);
        softmax's k-reduction rides the grad matmul via the tens column
        (10.0 column folds step_size=0.1 into the reciprocal)
      * update: DVE reciprocal+mul, gpsimd partition_broadcast
  - output projection through Wo columns in float32r, bf16 partial out
Host: transposes/casts inputs, sums the 8 bf16 partial outputs in fp32.
"""

import numpy as np
import ml_dtypes

BF16 = ml_dtypes.bfloat16

N_CORES = 8
D = 1024
K = 4096
Q = 2048
H = 16
HD = 64
STEPS = 5
STEP_SIZE = 0.1
BETA = 1.0 / np.sqrt(np.float32(HD))  # 1/8

QB = 512
LOG2E = 1.4426950408889634
SCH_DELTA = -0.0580  # tuned: mean-zero relative error of the bit-trick exp

# exp tile engine schedule per q-block: 32 k-chunks -> ACT or DVE.
# ACT is a bit faster per tile; DVE also does recip/mul side work.
N_DVE_TILES = 0
_dve_set = frozenset(
    int(round(i * 32.0 / N_DVE_TILES)) % 32 for i in range(N_DVE_TILES)
)

_CACHE = {}


def build_program(d=D, k=K, q=Q, steps=STEPS, n_cores=N_CORES,
                  n_dve=N_DVE_TILES, mode="bf16", gp_update=False):
    """Build + compile the per-core Bass program. Returns the Bacc object."""
    from contextlib import ExitStack

    import concourse.tile as tile
    from concourse import bacc, mybir

    f32 = mybir.dt.float32
    f32r = mybir.dt.float32r
    bf16 = mybir.dt.bfloat16
    fp8 = mybir.dt.float8e4
    i8 = mybir.dt.int8

    ndc = d // 128       # D chunks (contraction for projections)
    nkb = k // 512       # k blocks for K^T projection
    nkc = k // 128       # k chunks for the step loop
    nqb = q // QB        # q blocks
    beta = float(1.0 / np.sqrt(np.float64(HD)))
    if mode == "fp8dr":
        sch_c1 = beta * 8.0 * LOG2E
        sch_c2 = 8.0 * (7.0 + SCH_DELTA)
    else:
        sch_c1 = beta * 128.0 * LOG2E
        sch_c2 = 128.0 * (127.0 + SCH_DELTA)
    # pair-aligned engine assignment: kc pairs go entirely to one engine
    npd = n_dve // 2
    dve_pairs = frozenset(
        int(round(i * 16.0 / npd)) % 16 for i in range(npd)
    ) if npd else frozenset()
    dve_set = frozenset(x for p in dve_pairs for x in (2 * p, 2 * p + 1))
    nc = bacc.Bacc("TRN2", target_bir_lowering=False, debug=False,
                   num_devices=n_cores)
    ctxT = nc.dram_tensor("ctxT", [d, k], bf16, kind="ExternalInput").ap()
    tgtT = nc.dram_tensor("tgtT", [d, q], f32r, kind="ExternalInput").ap()
    wk = nc.dram_tensor("wk", [d, 128], bf16, kind="ExternalInput").ap()
    wq = nc.dram_tensor("wq", [d, 128], f32r, kind="ExternalInput").ap()
    woT = nc.dram_tensor("woT", [128, d], f32r, kind="ExternalInput").ap()
    out = nc.dram_tensor("out", [q, d], bf16, kind="ExternalOutput").ap()

    EXP = mybir.ActivationFunctionType.Exp
    MULT = mybir.AluOpType.mult
    ADD = mybir.AluOpType.add

    with tile.TileContext(nc) as tc, ExitStack() as ctx:
        # ---------------- persistent pools ----------------
        kt_pool = ctx.enter_context(tc.tile_pool(name="kt", bufs=1))
        kon_pool = ctx.enter_context(tc.tile_pool(name="kones", bufs=1))
        qt_pool = ctx.enter_context(tc.tile_pool(name="qt", bufs=2 * nqb))
        qtb_pool = ctx.enter_context(tc.tile_pool(name="qtb", bufs=2 * nqb))
        w_pool = ctx.enter_context(tc.tile_pool(name="w", bufs=1))

        if mode == "sdr":
            # fp8 K^T (pair layout) + DoubleRow-paired per-head copies:
            # kdr[h][c, j, k] = K^T[hd = 64h + c + 32j, k]
            kt8 = kt_pool.tile([128, k], fp8, tag="kt8")
            kdr = [kt_pool.tile([32, 2, k], fp8, tag=f"kdr{h}", name=f"kdr{h}")
                   for h in range(2)]
            ktp = None
        else:
            # per-head padded K^T: other head's rows zeroed -> full-128
            # contraction
            ktp = [kt_pool.tile([128, k], bf16, tag=f"ktp{h}", name=f"ktp{h}")
                   for h in range(2)]
            nc.vector.memset(ktp[0][64:128, :], 0.0)
            nc.vector.memset(ktp[1][0:64, :], 0.0)
        # K natural fp8, padded to 96: cols 0..63 = K chunk, col 64 = 10.0
        # (denominator), cols 65..95 = 0 (DoubleRow needs M % 32 == 0).
        kdt = fp8 if mode == "fp8dr" else bf16
        kones = [kon_pool.tile([128, nkc, 96], kdt, tag=f"kones{h}",
                               name=f"kones{h}")
                 for h in range(2)]
        for h in range(2):
            nc.vector.memset(kones[h][:, :, 64:96], 0.0)
            nc.gpsimd.memset(kones[h][:, :, 64:65], 10.0)
        if mode == "sdr":
            qdr = [kt_pool.tile([32, 2, q], fp8, tag=f"qdr{h}", name=f"qdr{h}")
                   for h in range(2)]
        wk_sb = w_pool.tile([128, d], bf16, tag="wk")
        wq_sb = w_pool.tile([128, d], f32r, tag="wq")
        wo_sb = w_pool.tile([128, d], f32r, tag="wo")

        for c in range(ndc):
            cs = slice(c * 128, (c + 1) * 128)
            nc.sync.dma_start(out=wk_sb[:, cs], in_=wk[cs, :])
            nc.sync.dma_start(out=wq_sb[:, cs], in_=wq[cs, :])
        nc.sync.dma_start(out=wo_sb[:], in_=woT[:])

        qt_tiles = []
        qtb_tiles = []

        # ---------------- phase A: projections ----------------
        with tc.tile_pool(name="ctxp", bufs=ndc) as ctx_pool, \
             tc.tile_pool(name="tgtp", bufs=ndc) as tgt_pool, \
             tc.tile_pool(name="psA", bufs=2, space="PSUM") as psA, \
             tc.tile_pool(name="psB", bufs=2, space="PSUM") as psB, \
             tc.tile_pool(name="psQ", bufs=2, space="PSUM") as psQ:
            ctx_tiles = [ctx_pool.tile([128, k], bf16, tag="ctx", name=f"ctx{c}")
                         for c in range(ndc)]
            tgt_tiles = [tgt_pool.tile([128, q], f32r, tag="tgt", name=f"tgt{c}")
                         for c in range(ndc)]
            for c in range(ndc):
                cs = slice(c * 128, (c + 1) * 128)
                nc.sync.dma_start(out=ctx_tiles[c][:], in_=ctxT[cs, :])
                nc.sync.dma_start(out=tgt_tiles[c][:], in_=tgtT[cs, :])

            # K^T = Wk_pair^T @ context^T  (bf16)
            for kb in range(nkb):
                ks = slice(kb * 512, (kb + 1) * 512)
                pk = psA.tile([128, 512], f32, tag="pk")
                for c in range(ndc):
                    cs = slice(c * 128, (c + 1) * 128)
                    nc.tensor.matmul(out=pk[:], lhsT=wk_sb[:, cs],
                                     rhs=ctx_tiles[c][:, ks],
                                     start=(c == 0), stop=(c == ndc - 1))
                if mode == "sdr":
                    nc.vector.tensor_copy(out=kt8[:, ks], in_=pk[:])
                else:
                    nc.vector.tensor_copy(out=ktp[0][0:64, ks],
                                          in_=pk[0:64, :])
                    nc.vector.tensor_copy(out=ktp[1][64:128, ks],
                                          in_=pk[64:128, :])

            if mode == "sdr":
                for h in range(2):
                    for jj in range(2):
                        base = 64 * h + 32 * jj
                        nc.sync.dma_start(out=kdr[h][:, jj, :],
                                          in_=kt8[base:base + 32, :])

            # K natural (both heads side by side), fp8 into kones
            for kc in range(nkc):
                ks = slice(kc * 128, (kc + 1) * 128)
                pn = psB.tile([128, 128], f32, tag="pn")
                for c in range(ndc):
                    cs = slice(c * 128, (c + 1) * 128)
                    nc.tensor.matmul(out=pn[:], lhsT=ctx_tiles[c][:, ks],
                                     rhs=wk_sb[:, cs],
                                     start=(c == 0), stop=(c == ndc - 1))
                for h in range(2):
                    nc.vector.tensor_copy(
                        out=kones[h][:, kc, 0:64],
                        in_=pn[:, h * 64:(h + 1) * 64])

            # Q^T projection in float32r (1 cyc/col at N=512)
            for j in range(nqb):
                qs = slice(j * QB, (j + 1) * QB)
                pq = psQ.tile([128, QB], f32, tag="pq")
                for c in range(ndc):
                    cs = slice(c * 128, (c + 1) * 128)
                    nc.tensor.matmul(out=pq[:], lhsT=wq_sb[:, cs],
                                     rhs=tgt_tiles[c][:, qs],
                                     start=(c == 0), stop=(c == ndc - 1))
                q0 = qt_pool.tile([128, QB], f32r, tag="qt")
                nc.vector.tensor_copy(out=q0[:], in_=pq[:])
                qdt = fp8 if mode == "sdr" else bf16
                qb0 = qtb_pool.tile([128, QB], qdt, tag="qtb")
                nc.scalar.copy(out=qb0[:], in_=pq[:])
                if mode == "sdr":
                    for h in range(2):
                        for jj in range(2):
                            base = 64 * h + 32 * jj
                            nc.sync.dma_start(
                                out=qdr[h][:, jj, j * QB:(j + 1) * QB],
                                in_=qb0[base:base + 32, :])
                qt_tiles.append(q0)
                qtb_tiles.append(qb0)

        # ---------------- phase B: energy steps ----------------
        with tc.tile_pool(name="pdrp", bufs=4) as pdr_pool, \
             tc.tile_pool(name="upd", bufs=3) as upd_pool, \
             tc.tile_pool(name="ps_s", bufs=3, space="PSUM") as ps_s, \
             tc.tile_pool(name="ps_g", bufs=1, space="PSUM") as ps_g:
            # p tiles: one per kc-pair [128, 2(kc), 2(h), QB]; pair-aligned
            # engine assignment keeps each tile single-writer-engine
            pdt = fp8 if mode == "fp8dr" else bf16
            for t in range(steps):
                new_qt = []
                new_qtb = []
                for j in range(nqb):
                    qcur = qt_tiles[j]
                    qbcur = qtb_tiles[j]
                    gt = ps_g.tile([96, 2, QB], f32, tag="g",
                                   name=f"g{t}_{j}")
                    ptile = None
                    for kc in range(nkc):
                        if kc % 2 == 0:
                            ptile = pdr_pool.tile([128, 2, 2, QB], pdt,
                                                  tag="pt")
                        s = ps_s.tile([128, 2 * QB], f32, tag="s")
                        for h in range(2):
                            if mode == "sdr":
                                nc.tensor.matmul(
                                    out=s[:, h * QB:(h + 1) * QB],
                                    lhsT=kdr[h][:, :, kc * 128:(kc + 1) * 128],
                                    rhs=qdr[h][:, :, j * QB:(j + 1) * QB],
                                    start=True, stop=True,
                                    perf_mode=mybir.MatmulPerfMode.DoubleRow)
                            else:
                                nc.tensor.matmul(
                                    out=s[:, h * QB:(h + 1) * QB],
                                    lhsT=ktp[h][:, kc * 128:(kc + 1) * 128],
                                    rhs=qbcur[:, :],
                                    start=True, stop=True)
                        pslice = ptile[:, kc % 2, :, :]
                        if kc in dve_set:
                            ibits = i8 if mode == "fp8dr" else mybir.dt.int16
                            nc.vector.tensor_scalar(
                                out=pslice.bitcast(ibits), in0=s[:],
                                scalar1=sch_c1, scalar2=sch_c2,
                                op0=MULT, op1=ADD)
                        else:
                            nc.scalar.activation(pslice, s[:], EXP, scale=beta)
                        if mode == "fp8dr" and kc % 2 == 1:
                            tpair = kc // 2
                            for h in range(2):
                                nc.tensor.matmul(
                                    out=gt[:, h, :],
                                    lhsT=kones[h][:, kc - 1:kc + 1, :],
                                    rhs=ptile[:, :, h, :],
                                    start=(tpair == 0), stop=(tpair == 15),
                                    perf_mode=mybir.MatmulPerfMode.DoubleRow)
                        elif mode != "fp8dr":
                            for h in range(2):
                                nc.tensor.matmul(
                                    out=gt[0:65, h, :],
                                    lhsT=kones[h][:, kc, 0:65],
                                    rhs=ptile[:, kc % 2, h, :],
                                    start=(kc == 0), stop=(kc == nkc - 1))
                    # q update: q += (G/10) / (denom/10) == q + 0.1*G/denom
                    # evacuate gt to SBUF first so the next q-block's grad
                    # matmuls can reuse the PSUM accumulator immediately
                    qn = qt_pool.tile([128, QB], f32r, tag="qt")
                    tm = upd_pool.tile([128, QB], f32, tag="tm")
                    t2 = upd_pool.tile([65, 2, QB], f32, tag="t2")
                    nc.vector.tensor_copy(out=t2[:], in_=gt[0:65, :, :])
                    for h in range(2):
                        hs = slice(h * 64, (h + 1) * 64)
                        # reciprocal lands on partition 0: partition_broadcast
                        # only reads correctly from a partition-0 source on HW
                        r = upd_pool.tile([1, QB], f32, tag="r")
                        nc.vector.reciprocal(out=r[:], in_=t2[64:65, h, :])
                        rb = upd_pool.tile([64, QB], f32, tag="rb")
                        nc.gpsimd.partition_broadcast(rb[:], r[0:1, :])
                        nc.vector.tensor_mul(out=tm[hs, :], in0=t2[0:64, h, :],
                                             in1=rb[:])
                    qdt = fp8 if mode == "sdr" else bf16
                    qb_new = qtb_pool.tile([128, QB], qdt, tag="qtb")
                    if gp_update:
                        nc.gpsimd.tensor_add(out=qn[:], in0=qcur[:], in1=tm[:])
                        nc.gpsimd.tensor_copy(out=qb_new[:], in_=qn[:])
                    else:
                        nc.vector.tensor_add(out=qn[:], in0=qcur[:], in1=tm[:])
                        nc.vector.tensor_copy(out=qb_new[:], in_=qn[:])
                    if mode == "sdr" and t < steps - 1:
                        for h in range(2):
                            for jj in range(2):
                                base = 64 * h + 32 * jj
                                nc.sync.dma_start(
                                    out=qdr[h][:, jj, j * QB:(j + 1) * QB],
                                    in_=qb_new[base:base + 32, :])
                    new_qt.append(qn)
                    new_qtb.append(qb_new)
                qt_tiles = new_qt
                qtb_tiles = new_qtb

        # ---------------- phase C: output projection (float32r) ----------------
        with tc.tile_pool(name="fo", bufs=6) as fo_pool, \
             tc.tile_pool(name="psO", bufs=4, space="PSUM") as psO:
            dob = min(512, d)
            for qb128 in range(q // 128):
                jt = qt_tiles[(qb128 * 128) // QB]
                qs = slice((qb128 * 128) % QB, (qb128 * 128) % QB + 128)
                for db in range(d // dob):
                    ds_ = slice(db * dob, (db + 1) * dob)
                    po = psO.tile([128, dob], f32, tag="po")
                    nc.tensor.matmul(out=po[:], lhsT=jt[:, qs],
                                     rhs=wo_sb[:, ds_],
                                     start=True, stop=True)
                    ot = fo_pool.tile([128, dob], bf16, tag="ot")
                    # alternate evacuation engines: ACT is idle in phase C
                    if db % 2 == 0:
                        nc.vector.tensor_copy(out=ot[:], in_=po[:])
                    else:
                        nc.scalar.copy(out=ot[:], in_=po[:])
                    nc.sync.dma_start(
                        out=out[qb128 * 128:(qb128 + 1) * 128, ds_],
                        in_=ot[:])

    nc.compile()
    return nc


def _get_program():
    if "nc" not in _CACHE:
        _CACHE["nc"] = build_program()
    return _CACHE["nc"]


def make_in_maps(context, target_init, Wq, Wk, Wo):
    """Host-side sharding/layout prep: one input map per core."""
    ctxT = np.ascontiguousarray(context.T).astype(BF16)        # [D, K]
    tgtT = np.ascontiguousarray(target_init.T.astype(np.float32))  # [D, Q]
    in_maps = []
    for c in range(N_CORES):
        h0, h1 = 2 * c, 2 * c + 1
        wk_c = np.concatenate([Wk[h0].T, Wk[h1].T], axis=1)    # [D, 128]
        wq_c = np.concatenate([Wq[h0].T, Wq[h1].T], axis=1)    # [D, 128]
        woT_c = np.ascontiguousarray(Wo[:, 128 * c:128 * (c + 1)].T)  # [128, D]
        in_maps.append({
            "ctxT": ctxT,
            "tgtT": tgtT,
            "wk": np.ascontiguousarray(wk_c).astype(BF16),
            "wq": np.ascontiguousarray(wq_c.astype(np.float32)),
            "woT": woT_c.astype(np.float32),
        })
    return in_maps


def kernel(context, target_init, Wq, Wk, Wo):
    context = np.asarray(context, dtype=np.float32)
    target_init = np.asarray(target_init, dtype=np.float32)
    Wq = np.asarray(Wq, dtype=np.float32)
    Wk = np.asarray(Wk, dtype=np.float32)
    Wo = np.asarray(Wo, dtype=np.float32)

    in_maps = make_in_maps(context, target_init, Wq, Wk, Wo)

    last_err = None
    for _attempt in range(3):
        try:
            results = _run_spmd(in_maps)
            break
        except Exception as e:  # transient axon RESOURCE_EXHAUSTED etc.
            last_err = e
            _CACHE.clear()
    else:
        raise last_err

    acc = np.zeros((Q, D), dtype=np.float32)
    for c in range(N_CORES):
        acc += results[c]["out"].astype(np.float32)
    return acc


def _run_spmd(in_maps):
    """Run the program on cores 0..7. Uses a cached jitted executable with
    device-resident zero buffers; falls back to run_bass_kernel_spmd."""
    nc = _get_program()
    try:
        runner = _CACHE.get("runner")
        if runner is None:
            runner = _SpmdRunner(nc, N_CORES)
            _CACHE["runner"] = runner
        return runner.run(in_maps)
    except Exception:
        _CACHE.pop("runner", None)
        from concourse.bass_utils import run_bass_kernel_spmd
        res = run_bass_kernel_spmd(nc, in_maps, list(range(N_CORES)))
        return res.results


class _SpmdRunner:
    """Persistent jitted shard_map executable (mirrors
    bass2jax.run_bass_via_pjrt's multi-core path, without output donation so
    the executable and zero buffers are reusable across calls)."""

    def __init__(self, nc, n_cores):
        import jax
        from jax.experimental.shard_map import shard_map
        from jax.sharding import Mesh, NamedSharding, PartitionSpec
        import concourse.mybir as mybir
        from concourse.bass2jax import (
            _bass_exec_p, install_neuronx_cc_hook, partition_id_tensor)

        install_neuronx_cc_hook()
        self.jax = jax
        self.n_cores = n_cores
        partition_name = (nc.partition_id_tensor.name
                          if nc.partition_id_tensor else None)
        in_names, out_names, out_avals, zero_outs = [], [], [], []
        for alloc in nc.m.functions[0].allocations:
            if not isinstance(alloc, mybir.MemoryLocationSet):
                continue
            name = alloc.memorylocations[0].name
            if alloc.kind == "ExternalInput":
                if name != partition_name:
                    in_names.append(name)
            elif alloc.kind == "ExternalOutput":
                shape = tuple(alloc.tensor_shape)
                dtype = mybir.dt.np(alloc.dtype)
                out_names.append(name)
                out_avals.append(jax.core.ShapedArray(shape, dtype))
                zero_outs.append(np.zeros(shape, dtype))
        self.in_names = in_names
        self.out_names = out_names
        self.out_avals = out_avals
        all_in_names = in_names + out_names
        if partition_name is not None:
            all_in_names.append(partition_name)

        def _body(*args):
            operands = list(args)
            if partition_name is not None:
                operands.append(partition_id_tensor())
            outs = _bass_exec_p.bind(
                *operands,
                out_avals=tuple(out_avals),
                in_names=tuple(all_in_names),
                out_names=tuple(out_names),
                lowering_input_output_aliases=(),
                sim_require_finite=True,
                sim_require_nnan=True,
                nc=nc,
            )
            return tuple(outs)

        devices = jax.devices()[:n_cores]
        mesh = Mesh(np.asarray(devices), ("core",))
        in_specs = (PartitionSpec("core"),) * (len(in_names) + len(out_names))
        out_specs = (PartitionSpec("core"),) * len(out_names)
        self.fn = jax.jit(
            shard_map(_body, mesh=mesh, in_specs=in_specs,
                      out_specs=out_specs, check_rep=False),
            keep_unused=True,
        )
        self.sharding = NamedSharding(mesh, PartitionSpec("core"))
        self.zeros_placed = [
            jax.device_put(np.concatenate([z] * n_cores, axis=0), self.sharding)
            for z in zero_outs
        ]

    def place(self, in_maps):
        concat = [
            np.concatenate([np.asarray(in_maps[c][n])
                            for c in range(self.n_cores)], axis=0)
            for n in self.in_names
        ]
        return [self.jax.device_put(a, self.sharding) for a in concat]

    def exec_placed(self, placed):
        outs = self.fn(*placed, *self.zeros_placed)
        self.jax.block_until_ready(outs)
        return outs

    def run(self, in_maps):
        outs = self.exec_placed(self.place(in_maps))
        per_core = []
        for c in range(self.n_cores):
            d = {}
            for i, n in enumerate(self.out_names):
                full = np.asarray(outs[i])
                sh = self.out_avals[i].shape
                d[n] = full.reshape(self.n_cores, *sh)[c]
            per_core.append(d)
        return per_core
